# revision 1
# baseline (speedup 1.0000x reference)
"""KAN block (2x KAN layer, dense_mlp) TRN2 Bass kernel — data-parallel on 8 cores.

Full inputs in, full output out. Tokens (B*S = 4096) are sharded 8 ways
(512 per core); weights are replicated.

Device math per KAN layer (out = silu(x) @ wb.T + einsum('nig,oig->no', B(x), ws)):
the 8 cubic B-spline bases B_g on the uniform 12-knot grid are expressed
exactly through 12 truncated-cube features

    a_j = relu(t_j - |x|)   (j = 6..11; knots t_0..t_5 < 0 never activate on |x|)
    u_j = a_j^3 * [x >= 0],   v_j = a_j^3 * [x < 0]

via B_g = sum_j Mu[g,j] u_j + Mv[g,j] v_j  (truncated-power representation of
B-splines evaluated from the near side, so feature magnitudes stay <= 2.2^3;
x outside the grid yields exact zeros on the feature side). The 8->12 map is
folded into the spline weights on the host, making each layer ONE dense
matmul with contraction over 13*I (silu + 12 spline features per input dim),
executed in fp32r (FP22 mantissa) at full PE speed with fp32 PSUM accumulation.

Layout: activations transposed (d on partitions, tokens on free dim), so
feature generation is elementwise on [128, 512] tiles and matmuls are
[128k x 128m]^T @ [128k x 512tok] -> PSUM [128m x 512tok].
"""

import numpy as np
from contextlib import ExitStack
from math import comb

import concourse.bass as bass
import concourse.bacc as bacc
import concourse.mybir as mybir
import concourse.tile as tile
from concourse.bass_utils import run_bass_kernel_spmd

F32 = mybir.dt.float32
F32R = mybir.dt.float32r
AF = mybir.ActivationFunctionType
ALU = mybir.AluOpType

# Problem constants (hardcoded per contract)
B, S, D, F = 2, 2048, 512, 2048
N_CORES = 8
T = (B * S) // N_CORES          # 512 tokens per core
G_INT, K_ORD = 5, 3
NKNOT = 12
NFEAT = 13                      # [silu, u6..u11, v6..v11]
ACT_CHAINS = (6, 7, 8, 9, 10)   # relu+square on ScalarE
DVE_CHAINS = (11,)              # full chain on VectorE (produces -a^3)
G1 = 4                          # layer-1 output tiles per PSUM group


def knots_f32():
    return (np.arange(-K_ORD, G_INT + K_ORD + 1, dtype=np.float32)
            * np.float32(2.0 / G_INT) - np.float32(1.0))


def fold_maps():
    inv6h3 = 1.0 / (6.0 * (2.0 / G_INT) ** 3)
    M = np.zeros((8, NKNOT))
    for g in range(8):
        for k in range(5):
            M[g, g + k] = ((-1) ** k) * comb(4, k) * inv6h3
    return M[:, 6:12].copy(), M[:, 5::-1].copy()


def fold_weights(wb, ws):
    """wb: (O, I), ws: (O, I, 8) -> (O, I, 13) fp32 augmented weights."""
    Mu, Mv = fold_maps()
    Wu = np.einsum('oig,gj->oij', ws.astype(np.float64), Mu)
    Wv = np.einsum('oig,gj->oij', ws.astype(np.float64), Mv)
    for j in DVE_CHAINS:
        Wu[:, :, j - 6] *= -1.0
        Wv[:, :, j - 6] *= -1.0
    Waug = np.concatenate([wb.astype(np.float64)[:, :, None], Wu, Wv], axis=2)
    return np.ascontiguousarray(Waug.astype(np.float32))


def pack_w1(Waug1):
    """(F, D, 13) -> (NG1, D_T*13, 128, G1*128): [mgroup, ktile, k_part, m_free]."""
    D_T, F_T = D // 128, F // 128
    NG1 = F_T // G1
    A = Waug1.reshape(NG1, G1 * 128, D_T, 128, NFEAT)
    A = A.transpose(0, 2, 4, 3, 1)
    return np.ascontiguousarray(A.reshape(NG1, D_T * NFEAT, 128, G1 * 128))


def pack_w2(Waug2):
    """(D, F, 13) -> (F_T, 128, 13, D): [d2group, k_part, feature, m_free]."""
    F_T = F // 128
    return np.ascontiguousarray(Waug2.transpose(1, 2, 0).reshape(F_T, 128, NFEAT, D))


def build_kernel():
    D_T, F_T = D // 128, F // 128
    NG1 = F_T // G1
    KT1, KT2 = D_T * NFEAT, F_T * NFEAT
    t = knots_f32()

    nc = bacc.Bacc()

    # knot constants as [128,1] const APs (activation bias operands)
    for j in range(6, 12):
        val = float(t[j])
        ctens = nc.alloc_sbuf_tensor(f"const-knot-{j}", [128, 1], F32)
        nc.gpsimd.memset(ctens.ap(), val)
        nc.const_aps.aps[(F32, val)] = ctens.ap()
    nc.all_engine_barrier()

    xT = nc.declare_dram_parameter("xT", [D, T], F32, isOutput=False)
    w1t = nc.declare_dram_parameter("w1t", [NG1, KT1, 128, G1 * 128], F32R,
                                    isOutput=False)
    w2t = nc.declare_dram_parameter("w2t", [F_T, 128, NFEAT, D], F32R,
                                    isOutput=False)
    outT = nc.declare_dram_parameter("outT", [D, T], F32, isOutput=True)

    with ExitStack() as ctx:
        tc = ctx.enter_context(tile.TileContext(nc))
        xpool = ctx.enter_context(tc.tile_pool(name="xp", bufs=1))
        f1pool = ctx.enter_context(tc.tile_pool(name="f1p", bufs=20))
        f2pool = ctx.enter_context(tc.tile_pool(name="f2p", bufs=26))
        scr = ctx.enter_context(tc.tile_pool(name="scr", bufs=3))
        w1pool = ctx.enter_context(tc.tile_pool(name="w1p", bufs=3))
        w2pool = ctx.enter_context(tc.tile_pool(name="w2p", bufs=2))
        opool = ctx.enter_context(tc.tile_pool(name="op", bufs=2))
        pp = ctx.enter_context(tc.tile_pool(name="pp", bufs=1, space="PSUM"))

        xtiles = []
        for dt in range(D_T):
            xt = xpool.tile([128, T], F32, name=f"x{dt}", tag=f"x{dt}")
            nc.sync.dma_start(out=xt, in_=xT[dt * 128:(dt + 1) * 128, :])
            xtiles.append(xt)

        psum2 = [pp.tile([128, T], F32, name=f"ps2_{m}", tag=f"l2psum{m}")
                 for m in range(D_T)]

        def gen_features(src, pool, blk):
            sig = scr.tile([128, T], F32, name=f"sig{blk}", tag="sig")
            nc.scalar.activation(sig, src, AF.Sigmoid)
            sil = pool.tile([128, T], F32R, name=f"sil{blk}", tag="feat")
            nc.vector.tensor_mul(sil, src, sig)
            y = scr.tile([128, T], F32, name=f"y{blk}", tag="y")
            nc.scalar.activation(y, src, AF.Abs)
            pos = scr.tile([128, T], F32, name=f"pos{blk}", tag="pos")
            nc.vector.tensor_scalar(out=pos, in0=src, scalar1=0.0, scalar2=None,
                                    op0=ALU.is_ge)
            us, vs = [], []
            for j in range(6, 12):
                tj = float(t[j])
                a = scr.tile([128, T], F32, name=f"a{blk}_{j}", tag="a")
                q = scr.tile([128, T], F32, name=f"q{blk}_{j}", tag="q")
                if j in ACT_CHAINS:
                    nc.scalar.activation(a, y, AF.Relu, bias=tj, scale=-1.0)
                    nc.scalar.activation(q, a, AF.Square)
                else:   # DVE chain: a = min(y - tj, 0) = -relu(tj - y)
                    nc.vector.tensor_scalar(out=a, in0=y, scalar1=tj,
                                            scalar2=0.0, op0=ALU.subtract,
                                            op1=ALU.min)
                    nc.vector.tensor_mul(q, a, a)
                c = scr.tile([128, T], F32, name=f"c{blk}_{j}", tag="c", bufs=4)
                nc.vector.tensor_mul(c, q, a)
                u = pool.tile([128, T], F32R, name=f"u{blk}_{j}", tag="feat")
                nc.vector.tensor_mul(u, c, pos)
                v = pool.tile([128, T], F32R, name=f"v{blk}_{j}", tag="feat")
                nc.gpsimd.tensor_sub(v, c, u)
                us.append(u)
                vs.append(v)
            return [sil] + us + vs

        for gm in range(NG1):
            psum1 = [pp.tile([128, T], F32, name=f"ps1_{gm}_{mi}",
                             tag=f"l1psum{mi}") for mi in range(G1)]
            for dt in range(D_T):
                feats = gen_features(xtiles[dt], f1pool, blk=f"a{gm}d{dt}")
                for f in range(NFEAT):
                    kt = dt * NFEAT + f
                    wt = w1pool.tile([128, G1 * 128], F32R,
                                     name=f"w1_{gm}_{kt}", tag="w1")
                    nc.sync.dma_start(out=wt, in_=w1t[gm, kt, :, :])
                    for mi in range(G1):
                        nc.tensor.matmul(
                            psum1[mi],
                            lhsT=wt[:, mi * 128:(mi + 1) * 128],
                            rhs=feats[f],
                            start=(kt == 0), stop=(kt == KT1 - 1),
                        )
            for mi in range(G1):
                g2 = gm * G1 + mi
                l2f = gen_features(psum1[mi], f2pool, blk=f"b{g2}")
                wt2a = w2pool.tile([128, 7, D], F32R, name=f"w2a_{g2}", tag="w2h")
                nc.sync.dma_start(out=wt2a, in_=w2t[g2, :, 0:7, :])
                wt2b = w2pool.tile([128, 6, D], F32R, name=f"w2b_{g2}", tag="w2h")
                nc.sync.dma_start(out=wt2b, in_=w2t[g2, :, 7:13, :])
                for f in range(NFEAT):
                    kt2 = g2 * NFEAT + f
                    wsl = wt2a[:, f, :] if f < 7 else wt2b[:, f - 7, :]
                    for m2 in range(D_T):
                        nc.tensor.matmul(
                            psum2[m2],
                            lhsT=wsl[:, m2 * 128:(m2 + 1) * 128],
                            rhs=l2f[f],
                            start=(kt2 == 0), stop=(kt2 == KT2 - 1),
                        )
        for m2 in range(D_T):
            ot = opool.tile([128, T], F32, name=f"o{m2}", tag="out")
            nc.scalar.activation(ot, psum2[m2], AF.Copy)
            nc.sync.dma_start(out=outT[m2 * 128:(m2 + 1) * 128, :], in_=ot)

    nc.finalize()
    return nc


_NC_CACHE = None


def _get_nc():
    global _NC_CACHE
    if _NC_CACHE is None:
        _NC_CACHE = build_kernel()
    return _NC_CACHE


def run(x, w1_base, w1_spline, w2_base, w2_spline, trace=False, **spmd_kwargs):
    x = np.asarray(x, dtype=np.float32)
    xf = np.ascontiguousarray(x.reshape(B * S, D))
    w1p = pack_w1(fold_weights(np.asarray(w1_base), np.asarray(w1_spline)))
    w2p = pack_w2(fold_weights(np.asarray(w2_base), np.asarray(w2_spline)))
    in_maps = []
    for c in range(N_CORES):
        shard = xf[c * T:(c + 1) * T]
        in_maps.append({
            "xT": np.ascontiguousarray(shard.T),
            "w1t": w1p,
            "w2t": w2p,
        })
    nc = _get_nc()
    res = run_bass_kernel_spmd(nc, in_maps, list(range(N_CORES)),
                               trace=trace, **spmd_kwargs)
    outs = [np.asarray(r["outT"]).T for r in res.results]   # each (T, D)
    out = np.concatenate(outs, axis=0).reshape(B, S, D).astype(np.float32)
    return out, res


def kernel(x, grid, w1_base, w1_spline, w2_base, w2_spline):
    out, _ = run(x, w1_base, w1_spline, w2_base, w2_spline)
    return out



# revision 8
# speedup vs baseline: 1.8368x; 1.8368x over previous
"""KAN block (2x KAN layer, dense_mlp) TRN2 Bass kernel — data-parallel on 8 cores.

Full inputs in, full output out. Tokens (B*S = 4096) are sharded 8 ways
(512 per core); weights are replicated.

Device math per KAN layer (out = silu(x) @ wb.T + einsum('nig,oig->no', B(x), ws)):
each cubic B-spline on the uniform grid obeys the exact 2-term identity

    B_g(x) = [ (2 - |s_g|)+^3 - 4 (1 - |s_g|)+^3 ] / 6,   s_g = (x - c_g)/h

with c_g the center knot t_{g+2}. The 16 "tent-cube" features (p_g^3, q_g^3)
are bounded (<= 8), vanish outside the grid automatically (matching the
reference's zero extrapolation), and have near-zero cancellation in the
contraction — so both features and spline weights quantize to fp8e4 and the
spline matmuls run in DoubleRow mode (2 k-rows/cycle). The silu base path
stays fp32r. Weights carry a x256 scale so fp8 weights sit in the e4m3 sweet
range; the scale is undone for free in activation affine slots.

Layout: activations transposed (d on partitions, tokens on free dim).
Feature chain per input tile (bf16 intermediates for 2x/4x DVE modes):
    xb = src * 1/(h*scale)            [DVE ts]
    m_g = |xb - c_g/h|                [ts, abs_max]
    np_g = min(m-2, 0) ; nq_g = min(m-1, 0)   -> pair tile [128,2,T]
    pq2  = Square(np|nq)              [ACT, one op per pair]
    f_g  = pq2 * npq -> fp8e4         [DVE TT, = (-p^3 | -q^3)]
Matmul pairs (lhsT [128,2,128], rhs [128,2,512]) accumulate with the fp32r
base matmuls in the same PSUM group.
"""

import numpy as np
import ml_dtypes
from contextlib import ExitStack

import concourse.bass as bass
import concourse.bacc as bacc
import concourse.mybir as mybir
import concourse.tile as tile
from concourse.bass_utils import run_bass_kernel_spmd

F32 = mybir.dt.float32
F32R = mybir.dt.float32r
BF16 = mybir.dt.bfloat16
FP8 = mybir.dt.float8e4
AF = mybir.ActivationFunctionType
ALU = mybir.AluOpType
DR = mybir.MatmulPerfMode.DoubleRow

# Problem constants (hardcoded per contract)
B, S, D, F = 2, 2048, 512, 2048
N_CORES = 8
T = (B * S) // N_CORES          # 512 tokens per core
G_INT = 5
H = 2.0 / G_INT                 # 0.4 knot spacing
NP = 8                          # 8 (p,q) tent pairs = 8 B-splines
NG1 = 4                         # layer-1 psum groups (4 m-tiles each)
D_T, F_T = D // 128, F // 128   # 4, 16
SC = 256.0                      # weight scale (psum carries SC*value)
CT = [float(g) - 3.5 for g in range(NP)]   # centers / h


def _e4(a):
    return np.clip(a, -240.0, 240.0).astype(ml_dtypes.float8_e4m3)


def pack_l1(w1b, w1s):
    """w1b (F,D), w1s (F,D,8) ->
       w1q (NG1, D_T, 128, NP, 2, 512) fp8, w1bt (NG1, D_T, 128, 512) f32."""
    A = np.asarray(w1s, np.float64).reshape(NG1, 4 * 128, D_T, 128, NP)
    A = A.transpose(0, 2, 3, 4, 1)                     # (gm, dt, i, g, o)
    w1q = _e4(np.stack([(-SC / 6.0) * A, (SC * 4.0 / 6.0) * A], axis=4))
    Wb = np.asarray(w1b, np.float64).reshape(NG1, 4 * 128, D_T, 128)
    Wb = (SC * Wb).transpose(0, 2, 3, 1)               # (gm, dt, i, o)
    return np.ascontiguousarray(w1q), np.ascontiguousarray(Wb.astype(np.float32))


def pack_l2(w2b, w2s):
    """w2b (D,F), w2s (D,F,8) ->
       w2q (F_T, 128, NP, 2, 512) fp8, w2bt (F_T, 128, 512) f32."""
    A = np.asarray(w2s, np.float64).reshape(D, F_T, 128, NP)
    A = A.transpose(1, 2, 3, 0)                        # (g2, i, g, o)
    w2q = _e4(np.stack([(-SC / 6.0) * A, (SC * 4.0 / 6.0) * A], axis=3))
    Wb = np.asarray(w2b, np.float64).reshape(D, F_T, 128)
    Wb = (SC * Wb).transpose(1, 2, 0)                  # (g2, i, o)
    return np.ascontiguousarray(w2q), np.ascontiguousarray(Wb.astype(np.float32))


def build_kernel():
    nc = bacc.Bacc()

    # const APs for activation bias operands (0.0 for Silu/Square, -center
    # for the Abs ops)
    for val in [0.0] + [-c for c in CT]:
        ctens = nc.alloc_sbuf_tensor(f"const-{val}", [128, 1], F32)
        nc.gpsimd.memset(ctens.ap(), float(val))
        nc.const_aps.aps[(F32, float(val))] = ctens.ap()
    nc.all_engine_barrier()

    xT = nc.declare_dram_parameter("xT", [D, T], F32, isOutput=False)
    w1q = nc.declare_dram_parameter("w1q", [NG1, D_T, 128, NP, 2, 512], FP8,
                                    isOutput=False)
    w1bt = nc.declare_dram_parameter("w1bt", [NG1, D_T, 128, 512], F32R,
                                     isOutput=False)
    w2q = nc.declare_dram_parameter("w2q", [F_T, 128, NP, 2, 512], FP8,
                                    isOutput=False)
    w2bt = nc.declare_dram_parameter("w2bt", [F_T, 128, 512], F32R,
                                     isOutput=False)
    outT = nc.declare_dram_parameter("outT", [D, T], F32, isOutput=True)

    with ExitStack() as ctx:
        tc = ctx.enter_context(tile.TileContext(nc))
        xp = ctx.enter_context(tc.tile_pool(name="xp", bufs=1))
        f1p = ctx.enter_context(tc.tile_pool(name="f1p", bufs=1))
        f2p = ctx.enter_context(tc.tile_pool(name="f2p", bufs=1))
        scr = ctx.enter_context(tc.tile_pool(name="scr", bufs=4))
        w1pool = ctx.enter_context(tc.tile_pool(name="w1p", bufs=3))
        w2pool = ctx.enter_context(tc.tile_pool(name="w2p", bufs=3))
        opool = ctx.enter_context(tc.tile_pool(name="op", bufs=2))
        pp = ctx.enter_context(tc.tile_pool(name="pp", bufs=1, space="PSUM"))

        def gen_features(src, fpool, blk, siltag, ftagpfx, l2, silbufs=1,
                         fbufs=1):
            """src: [128,T] fp32 (SBUF or PSUM). Returns (sil, [8 pair tiles])."""
            sil = fpool.tile([128, T], F32R, name=f"sil{blk}", tag=siltag,
                             bufs=silbufs)
            nc.scalar.activation(sil, src, AF.Silu,
                                 scale=(1.0 / SC) if l2 else 1.0)
            xb = scr.tile([128, T], BF16, name=f"xb{blk}", tag="xb")
            nc.vector.tensor_scalar(out=xb, in0=src,
                                    scalar1=(1.0 / (SC * H)) if l2 else (1.0 / H),
                                    scalar2=None, op0=ALU.mult)
            fpairs = []
            for g in range(NP):
                m = scr.tile([128, T], BF16, name=f"m{blk}_{g}", tag="m")
                nc.scalar.activation(m, xb, AF.Abs, bias=-CT[g])
                npq = scr.tile([128, 2, T], BF16, name=f"npq{blk}_{g}", tag="npq")
                nc.vector.tensor_scalar(out=npq[:, 0, :], in0=m, scalar1=2.0,
                                        scalar2=0.0, op0=ALU.subtract,
                                        op1=ALU.min)
                nc.vector.tensor_scalar(out=npq[:, 1, :], in0=m, scalar1=1.0,
                                        scalar2=0.0, op0=ALU.subtract,
                                        op1=ALU.min)
                pq2 = scr.tile([128, 2, T], BF16, name=f"pq2{blk}_{g}", tag="pq2")
                nc.scalar.activation(pq2, npq, AF.Square)
                fp = fpool.tile([128, 2, T], FP8, name=f"f{blk}_{g}",
                                tag=f"{ftagpfx}_{g}", bufs=fbufs)
                nc.vector.tensor_mul(fp, pq2, npq)
                fpairs.append(fp)
            return sil, fpairs

        # ---- load x, generate layer-1 features (once) ----
        xtiles = []
        for dt in range(D_T):
            xt = xp.tile([128, T], F32, name=f"x{dt}", tag=f"x{dt}")
            nc.sync.dma_start(out=xt, in_=xT[dt * 128:(dt + 1) * 128, :])
            xtiles.append(xt)

        sil1, f1 = [], []
        for dt in range(D_T):
            s, fp = gen_features(xtiles[dt], f1p, blk=f"a{dt}",
                                 siltag=f"sil1_{dt}", ftagpfx=f"f1_{dt}",
                                 l2=False)
            sil1.append(s)
            f1.append(fp)

        psum2 = [pp.tile([128, T], F32, name=f"ps2_{m2}", tag=f"ps2_{m2}")
                 for m2 in range(D_T)]

        def emit_l1(gm):
            ps = [pp.tile([128, T], F32, name=f"ps1_{gm}_{mi}", tag=f"ps1_{mi}")
                  for mi in range(4)]
            for dt in range(D_T):
                wb = w1pool.tile([128, 512], F32R, name=f"w1b_{gm}_{dt}",
                                 tag="w1b")
                nc.sync.dma_start(out=wb, in_=w1bt[gm, dt])
                for mi in range(4):
                    nc.tensor.matmul(ps[mi],
                                     lhsT=wb[:, mi * 128:(mi + 1) * 128],
                                     rhs=sil1[dt], start=(dt == 0), stop=False)
            for dt in range(D_T):
                wq = w1pool.tile([128, NP, 2, 512], FP8, name=f"w1q_{gm}_{dt}",
                                 tag="w1q")
                nc.sync.dma_start(out=wq, in_=w1q[gm, dt])
                for g in range(NP):
                    for mi in range(4):
                        nc.tensor.matmul(
                            ps[mi],
                            lhsT=wq[:, g, :, mi * 128:(mi + 1) * 128],
                            rhs=f1[dt][g], perf_mode=DR,
                            start=False,
                            stop=(dt == D_T - 1 and g == NP - 1))
            return ps

        def emit_l2(gm, sil2, f2):
            for mi in range(4):
                g2 = gm * 4 + mi
                wb = w2pool.tile([128, 512], F32R, name=f"w2b_{g2}", tag="w2b")
                nc.sync.dma_start(out=wb, in_=w2bt[g2])
                for m2 in range(D_T):
                    nc.tensor.matmul(psum2[m2],
                                     lhsT=wb[:, m2 * 128:(m2 + 1) * 128],
                                     rhs=sil2[mi], start=(g2 == 0), stop=False)
                wq = w2pool.tile([128, NP, 2, 512], FP8, name=f"w2q_{g2}",
                                 tag="w2q")
                nc.sync.dma_start(out=wq, in_=w2q[g2])
                for g in range(NP):
                    for m2 in range(D_T):
                        nc.tensor.matmul(
                            psum2[m2],
                            lhsT=wq[:, g, :, m2 * 128:(m2 + 1) * 128],
                            rhs=f2[mi][g], perf_mode=DR,
                            start=False,
                            stop=(g2 == F_T - 1 and g == NP - 1))

        # ---- main pipeline: L1(gm) matmuls || L2 feature-gen || L2 matmuls ----
        psum1 = emit_l1(0)
        for gm in range(NG1):
            sil2, f2 = [], []
            for mi in range(4):
                g2 = gm * 4 + mi
                s, fp = gen_features(psum1[mi], f2p, blk=f"b{g2}",
                                     siltag=f"sil2_{mi}", ftagpfx=f"f2_{mi}",
                                     l2=True, silbufs=2)
                sil2.append(s)
                f2.append(fp)
            if gm < NG1 - 1:
                psum1 = emit_l1(gm + 1)
            emit_l2(gm, sil2, f2)

        for m2 in range(D_T):
            ot = opool.tile([128, T], F32, name=f"o{m2}", tag="out")
            nc.scalar.activation(ot, psum2[m2], AF.Copy, scale=1.0 / SC)
            nc.sync.dma_start(out=outT[m2 * 128:(m2 + 1) * 128, :], in_=ot)

    nc.finalize()
    return nc


_NC_CACHE = None


def _get_nc():
    global _NC_CACHE
    if _NC_CACHE is None:
        _NC_CACHE = build_kernel()
    return _NC_CACHE


def run(x, w1_base, w1_spline, w2_base, w2_spline, trace=False, **spmd_kwargs):
    x = np.asarray(x, dtype=np.float32)
    xf = np.ascontiguousarray(x.reshape(B * S, D))
    w1qa, w1ba = pack_l1(np.asarray(w1_base), np.asarray(w1_spline))
    w2qa, w2ba = pack_l2(np.asarray(w2_base), np.asarray(w2_spline))
    in_maps = []
    for c in range(N_CORES):
        shard = xf[c * T:(c + 1) * T]
        in_maps.append({
            "xT": np.ascontiguousarray(shard.T),
            "w1q": w1qa,
            "w1bt": w1ba,
            "w2q": w2qa,
            "w2bt": w2ba,
        })
    nc = _get_nc()
    res = run_bass_kernel_spmd(nc, in_maps, list(range(N_CORES)),
                               trace=trace, **spmd_kwargs)
    outs = [np.asarray(r["outT"]).T for r in res.results]   # each (T, D)
    out = np.concatenate(outs, axis=0).reshape(B, S, D).astype(np.float32)
    return out, res


def kernel(x, grid, w1_base, w1_spline, w2_base, w2_spline):
    out, _ = run(x, w1_base, w1_spline, w2_base, w2_spline)
    return out


# revision 14
# speedup vs baseline: 2.7888x; 1.5183x over previous
"""KAN block (2x KAN layer, dense_mlp) TRN2 Bass kernel — data-parallel on 8 cores.

Full inputs in, full output out. Tokens (B*S = 4096) are sharded 8 ways
(512 per core); weights are replicated.

Device math per KAN layer (out = silu(x) @ wb.T + einsum('nig,oig->no', B(x), ws)):
each cubic B-spline on the uniform grid obeys the exact 2-term identity

    B_g(x) = [ (2 - |s_g|)+^3 - 4 (1 - |s_g|)+^3 ] / 6,   s_g = (x - c_g)/h

with c_g the center knot t_{g+2}. The 16 "tent-cube" features (p_g^3, q_g^3)
are bounded (<= 8), vanish outside the grid automatically (matching the
reference's zero extrapolation), and have near-zero cancellation in the
contraction — so both features and spline weights quantize to fp8e4 and the
spline matmuls run in DoubleRow mode (2 k-rows/cycle). The silu base path
stays fp32r. Weights carry a x256 scale so fp8 weights sit in the e4m3 sweet
range; the scale is undone for free in activation affine slots.

Layout: activations transposed (d on partitions, tokens on free dim).
Feature chain per input tile (bf16 intermediates for 2x/4x DVE modes):
    xb = src * 1/(h*scale)            [DVE ts]
    m_g = |xb - c_g/h|                [ts, abs_max]
    np_g = min(m-2, 0) ; nq_g = min(m-1, 0)   -> pair tile [128,2,T]
    pq2  = Square(np|nq)              [ACT, one op per pair]
    f_g  = pq2 * npq -> fp8e4         [DVE TT, = (-p^3 | -q^3)]
Matmul pairs (lhsT [128,2,128], rhs [128,2,512]) accumulate with the fp32r
base matmuls in the same PSUM group.
"""

import numpy as np
import ml_dtypes
from contextlib import ExitStack

import concourse.bass as bass
import concourse.bacc as bacc
import concourse.mybir as mybir
import concourse.tile as tile
import concourse.dve_ops as dve_ops
from concourse.bass_utils import run_bass_kernel_spmd
from concourse.dve_spec import (
    C0, C1, C2, AluOp as DveAlu, Bin, Spec, Src0, Zero, maxx, minn, sq,
)


def _register_tent_cube():
    """Custom fused DVE op: out = t^3, t = min(|in0 - s0|*imm2 + s1, 0).

    Computes a full tent-cube feature (-p^3 with p = (|s|+s1)_- clamp) in a
    single Vector pass, replacing a 5-op chain across two engines."""
    name = "TENT_CUBE_ANT"
    if name in dve_ops._SUB_OPCODE_FOR_NAME:
        return next(op for op in dve_ops.OPS if op.name == name)
    d = Bin(DveAlu.SUBTRACT, Src0, C0)
    nd = Bin(DveAlu.SUBTRACT, C0, Src0)
    t = minn(Bin(DveAlu.ADD, Bin(DveAlu.MULTIPLY, maxx(d, nd), C2), C1), Zero)
    op = dve_ops.DveOp(
        name,
        Spec(
            body=Bin(DveAlu.MULTIPLY, sq(t), t),
            reference=lambda in0, in1, s0, s1, imm2: (
                np.minimum(np.abs(in0.astype(np.float32) - s0) * imm2 + s1,
                           0.0) ** 3
            ),
        ),
        subdim=False,
        uops_sha={"v3": "8ebf8b8de8801ea4", "v4": "881f6604669d5cbb"},
    )
    dve_ops.OPS.append(op)
    dve_ops._SUB_OPCODE_FOR_NAME[name] = (
        dve_ops._CUSTOM_DVE_ROW_BASE + len(dve_ops.OPS) - 1
    )
    assert dve_ops._SUB_OPCODE_FOR_NAME[name] < 0x20
    return op


TENT_CUBE = _register_tent_cube()

F32 = mybir.dt.float32
F32R = mybir.dt.float32r
BF16 = mybir.dt.bfloat16
FP8 = mybir.dt.float8e4
AF = mybir.ActivationFunctionType
ALU = mybir.AluOpType
DR = mybir.MatmulPerfMode.DoubleRow

# Problem constants (hardcoded per contract)
B, S, D, F = 2, 2048, 512, 2048
N_CORES = 8
T = (B * S) // N_CORES          # 512 tokens per core
G_INT = 5
H = 2.0 / G_INT                 # 0.4 knot spacing
NP = 8                          # 8 (p,q) tent pairs = 8 B-splines
NG1 = 4                         # layer-1 psum groups (4 m-tiles each)
D_T, F_T = D // 128, F // 128   # 4, 16
SC = 256.0                      # weight scale (psum carries SC*value)
CT = [float(g) - 3.5 for g in range(NP)]   # centers / h


def _e4(a):
    return np.clip(a, -240.0, 240.0).astype(ml_dtypes.float8_e4m3)


def pack_l1(w1b, w1s):
    """w1b (F,D), w1s (F,D,8) ->
       w1q (NG1, D_T, 128, NP, 2, 512) fp8, w1bt (NG1, D_T, 128, 512) f32."""
    A = np.asarray(w1s, np.float64).reshape(NG1, 4 * 128, D_T, 128, NP)
    A = A.transpose(0, 2, 3, 4, 1)                     # (gm, dt, i, g, o)
    w1q = _e4(np.stack([(-SC / 6.0) * A, (SC * 4.0 / 6.0) * A], axis=4))
    Wb = np.asarray(w1b, np.float64).reshape(NG1, 4 * 128, D_T, 128)
    Wb = (SC * Wb).transpose(0, 2, 3, 1)               # (gm, dt, i, o)
    return np.ascontiguousarray(w1q), np.ascontiguousarray(Wb.astype(np.float32))


def pack_l2(w2b, w2s):
    """w2b (D,F), w2s (D,F,8) ->
       w2q (F_T, 128, NP, 2, 512) fp8, w2bt (F_T, 128, 512) f32."""
    A = np.asarray(w2s, np.float64).reshape(D, F_T, 128, NP)
    A = A.transpose(1, 2, 3, 0)                        # (g2, i, g, o)
    w2q = _e4(np.stack([(-SC / 6.0) * A, (SC * 4.0 / 6.0) * A], axis=3))
    Wb = np.asarray(w2b, np.float64).reshape(D, F_T, 128)
    Wb = (SC * Wb).transpose(1, 2, 0)                  # (g2, i, o)
    return np.ascontiguousarray(w2q), np.ascontiguousarray(Wb.astype(np.float32))


def build_kernel():
    nc = bacc.Bacc()

    # const APs for activation bias operands (0.0 for Silu/Square, -center
    # for the Abs ops)
    for val in [0.0] + [-c for c in CT]:
        ctens = nc.alloc_sbuf_tensor(f"const-{val}", [128, 1], F32)
        nc.gpsimd.memset(ctens.ap(), float(val))
        nc.const_aps.aps[(F32, float(val))] = ctens.ap()
    nc.all_engine_barrier()

    xT = nc.declare_dram_parameter("xT", [D, T], F32, isOutput=False)
    w1q = nc.declare_dram_parameter("w1q", [NG1, D_T, 128, NP, 2, 512], FP8,
                                    isOutput=False)
    w1bt = nc.declare_dram_parameter("w1bt", [NG1, D_T, 128, 512], F32R,
                                     isOutput=False)
    w2q = nc.declare_dram_parameter("w2q", [F_T, 128, NP, 2, 512], FP8,
                                    isOutput=False)
    w2bt = nc.declare_dram_parameter("w2bt", [F_T, 128, 512], F32R,
                                     isOutput=False)
    outT = nc.declare_dram_parameter("outT", [D, T], F32, isOutput=True)

    with ExitStack() as ctx:
        tc = ctx.enter_context(tile.TileContext(nc))
        xp = ctx.enter_context(tc.tile_pool(name="xp", bufs=1))
        f1p = ctx.enter_context(tc.tile_pool(name="f1p", bufs=1))
        f2p = ctx.enter_context(tc.tile_pool(name="f2p", bufs=1))
        scr = ctx.enter_context(tc.tile_pool(name="scr", bufs=7))
        w1pool = ctx.enter_context(tc.tile_pool(name="w1p", bufs=3))
        w2pool = ctx.enter_context(tc.tile_pool(name="w2p", bufs=3))
        opool = ctx.enter_context(tc.tile_pool(name="op", bufs=1))
        pp = ctx.enter_context(tc.tile_pool(name="pp", bufs=1, space="PSUM"))

        def gen_features(src, fpool, blk, siltag, ftagpfx, l2, silbufs=1,
                         fbufs=1):
            """src: [128,T] fp32 (SBUF or PSUM). Returns (sil, [8 pair tiles])."""
            sil = fpool.tile([128, T], F32R, name=f"sil{blk}", tag=siltag,
                             bufs=silbufs)
            nc.scalar.activation(sil, src, AF.Silu,
                                 scale=(1.0 / SC) if l2 else 1.0)
            if l2:
                # one short-lived PSUM read so the bank frees early; the 16
                # tent ops below read the bf16 copy
                xb = scr.tile([128, T], BF16, name=f"xb{blk}", tag="xb")
                nc.vector.tensor_scalar(out=xb, in0=src,
                                        scalar1=1.0 / (SC * H), scalar2=None,
                                        op0=ALU.mult)
                tsrc, s0s, inv = xb, CT, 1.0
            else:
                tsrc, s0s, inv = src, [c * H for c in CT], 1.0 / H
            fpairs = []
            for g in range(NP):
                fp = fpool.tile([128, 2, T], FP8, name=f"f{blk}_{g}",
                                tag=f"{ftagpfx}_{g}", bufs=fbufs)
                nc.vector._custom_dve(TENT_CUBE, out=fp[:, 0, :], in0=tsrc,
                                      s0=float(s0s[g]), s1=-2.0, imm2=inv)
                nc.vector._custom_dve(TENT_CUBE, out=fp[:, 1, :], in0=tsrc,
                                      s0=float(s0s[g]), s1=-1.0, imm2=inv)
                fpairs.append(fp)
            return sil, fpairs

        # ---- load x, generate layer-1 features (once) ----
        xtiles = []
        for dt in range(D_T):
            xt = xp.tile([128, T], F32, name=f"x{dt}", tag=f"x{dt}")
            nc.sync.dma_start(out=xt, in_=xT[dt * 128:(dt + 1) * 128, :])
            xtiles.append(xt)

        sil1, f1 = [], []
        for dt in range(D_T):
            s, fp = gen_features(xtiles[dt], f1p, blk=f"a{dt}",
                                 siltag=f"sil1_{dt}", ftagpfx=f"f1_{dt}",
                                 l2=False)
            sil1.append(s)
            f1.append(fp)

        psum2 = [pp.tile([128, T], F32, name=f"ps2_{m2}", tag=f"ps2_{m2}")
                 for m2 in range(D_T)]

        def emit_l1(gm):
            ps = [pp.tile([128, T], F32, name=f"ps1_{gm}_{mi}", tag=f"ps1_{mi}")
                  for mi in range(4)]
            for dt in range(D_T):
                wb = w1pool.tile([128, 512], F32R, name=f"w1b_{gm}_{dt}",
                                 tag="w1b")
                nc.sync.dma_start(out=wb, in_=w1bt[gm, dt])
                for mi in range(4):
                    nc.tensor.matmul(ps[mi],
                                     lhsT=wb[:, mi * 128:(mi + 1) * 128],
                                     rhs=sil1[dt], start=(dt == 0), stop=False)
            for dt in range(D_T):
                wq = w1pool.tile([128, NP, 2, 512], FP8, name=f"w1q_{gm}_{dt}",
                                 tag="w1q")
                nc.sync.dma_start(out=wq, in_=w1q[gm, dt])
                for g in range(NP):
                    for mi in range(4):
                        nc.tensor.matmul(
                            ps[mi],
                            lhsT=wq[:, g, :, mi * 128:(mi + 1) * 128],
                            rhs=f1[dt][g], perf_mode=DR,
                            start=False,
                            stop=(dt == D_T - 1 and g == NP - 1))
            return ps

        def emit_l2(gm, sil2, f2):
            for mi in range(4):
                g2 = gm * 4 + mi
                wb = w2pool.tile([128, 512], F32R, name=f"w2b_{g2}", tag="w2b")
                nc.sync.dma_start(out=wb, in_=w2bt[g2])
                for m2 in range(D_T):
                    nc.tensor.matmul(psum2[m2],
                                     lhsT=wb[:, m2 * 128:(m2 + 1) * 128],
                                     rhs=sil2[mi], start=(g2 == 0), stop=False)
                wq = w2pool.tile([128, NP, 2, 512], FP8, name=f"w2q_{g2}",
                                 tag="w2q")
                nc.sync.dma_start(out=wq, in_=w2q[g2])
                for g in range(NP):
                    for m2 in range(D_T):
                        nc.tensor.matmul(
                            psum2[m2],
                            lhsT=wq[:, g, :, m2 * 128:(m2 + 1) * 128],
                            rhs=f2[mi][g], perf_mode=DR,
                            start=False,
                            stop=(g2 == F_T - 1 and g == NP - 1))

        # ---- main pipeline: L1(gm) matmuls || L2 feature-gen || L2 matmuls ----
        psum1 = emit_l1(0)
        for gm in range(NG1):
            sil2, f2 = [], []
            for mi in range(4):
                g2 = gm * 4 + mi
                s, fp = gen_features(psum1[mi], f2p, blk=f"b{g2}",
                                     siltag=f"sil2_{mi}", ftagpfx=f"f2_{mi}",
                                     l2=True, silbufs=2)
                sil2.append(s)
                f2.append(fp)
            if gm < NG1 - 1:
                psum1 = emit_l1(gm + 1)
            emit_l2(gm, sil2, f2)

        for m2 in range(D_T):
            ot = opool.tile([128, T], F32, name=f"o{m2}", tag="out")
            nc.scalar.activation(ot, psum2[m2], AF.Copy, scale=1.0 / SC)
            nc.sync.dma_start(out=outT[m2 * 128:(m2 + 1) * 128, :], in_=ot)

    nc.finalize()
    return nc


_NC_CACHE = None


def _get_nc():
    global _NC_CACHE
    if _NC_CACHE is None:
        _NC_CACHE = build_kernel()
    return _NC_CACHE


def run(x, w1_base, w1_spline, w2_base, w2_spline, trace=False, **spmd_kwargs):
    x = np.asarray(x, dtype=np.float32)
    xf = np.ascontiguousarray(x.reshape(B * S, D))
    w1qa, w1ba = pack_l1(np.asarray(w1_base), np.asarray(w1_spline))
    w2qa, w2ba = pack_l2(np.asarray(w2_base), np.asarray(w2_spline))
    in_maps = []
    for c in range(N_CORES):
        shard = xf[c * T:(c + 1) * T]
        in_maps.append({
            "xT": np.ascontiguousarray(shard.T),
            "w1q": w1qa,
            "w1bt": w1ba,
            "w2q": w2qa,
            "w2bt": w2ba,
        })
    nc = _get_nc()
    res = run_bass_kernel_spmd(nc, in_maps, list(range(N_CORES)),
                               trace=trace, **spmd_kwargs)
    outs = [np.asarray(r["outT"]).T for r in res.results]   # each (T, D)
    out = np.concatenate(outs, axis=0).reshape(B, S, D).astype(np.float32)
    return out, res


def kernel(x, grid, w1_base, w1_spline, w2_base, w2_spline):
    out, _ = run(x, w1_base, w1_spline, w2_base, w2_spline)
    return out


# revision 18
# speedup vs baseline: 2.8128x; 1.0086x over previous
"""KAN block (2x KAN layer, dense_mlp) TRN2 Bass kernel — data-parallel on 8 cores.

Full inputs in, full output out. Tokens (B*S = 4096) are sharded 8 ways
(512 per core); weights are replicated.

Device math per KAN layer (out = silu(x) @ wb.T + einsum('nig,oig->no', B(x), ws)):
each cubic B-spline on the uniform grid obeys the exact 2-term identity

    B_g(x) = [ (2 - |s_g|)+^3 - 4 (1 - |s_g|)+^3 ] / 6,   s_g = (x - c_g)/h

with c_g the center knot t_{g+2}. The 16 "tent-cube" features (p_g^3, q_g^3)
are bounded (<= 8), vanish outside the grid automatically (matching the
reference's zero extrapolation), and have near-zero cancellation in the
contraction — so both features and spline weights quantize to fp8e4 and the
spline matmuls run in DoubleRow mode (2 k-rows/cycle). The silu base path
stays fp32r. Weights carry a x256 scale so fp8 weights sit in the e4m3 sweet
range; the scale is undone for free in activation affine slots.

Layout: activations transposed (d on partitions, tokens on free dim).
Each tent-cube feature is ONE fused custom-DVE op (registered at import
time into concourse's custom-DVE table):

    TENT_CUBE_ANT: out = t^3,  t = min(|in0 - s0|*imm2 + s1, 0)   -> fp8e4

so a feature costs a single Vector pass (~0.6us/tile) instead of a 5-op
chain across ScalarE+VectorE. Matmul pairs (lhsT [128,2,128], rhs
[128,2,512]) accumulate with the fp32r base matmuls in the same PSUM group.
The only remaining ScalarE work is Silu and the final psum copies.
"""

import numpy as np
import ml_dtypes
from contextlib import ExitStack

import concourse.bass as bass
import concourse.bacc as bacc
import concourse.mybir as mybir
import concourse.tile as tile
import concourse.dve_ops as dve_ops
from concourse.bass_utils import run_bass_kernel_spmd
from concourse.dve_spec import (
    C0, C1, C2, AluOp as DveAlu, Bin, Spec, Src0, Zero, maxx, minn, sq,
)


def _register_tent_cube():
    """Custom fused DVE op: out = t^3, t = min(|in0 - s0|*imm2 + s1, 0).

    Computes a full tent-cube feature (-p^3 with p = (|s|+s1)_- clamp) in a
    single Vector pass, replacing a 5-op chain across two engines."""
    name = "TENT_CUBE_ANT"
    if name in dve_ops._SUB_OPCODE_FOR_NAME:
        return next(op for op in dve_ops.OPS if op.name == name)
    d = Bin(DveAlu.SUBTRACT, Src0, C0)
    nd = Bin(DveAlu.SUBTRACT, C0, Src0)
    t = minn(Bin(DveAlu.ADD, Bin(DveAlu.MULTIPLY, maxx(d, nd), C2), C1), Zero)
    op = dve_ops.DveOp(
        name,
        Spec(
            body=Bin(DveAlu.MULTIPLY, sq(t), t),
            reference=lambda in0, in1, s0, s1, imm2: (
                np.minimum(np.abs(in0.astype(np.float32) - s0) * imm2 + s1,
                           0.0) ** 3
            ),
        ),
        subdim=False,
        uops_sha={"v3": "8ebf8b8de8801ea4", "v4": "881f6604669d5cbb"},
    )
    dve_ops.OPS.append(op)
    dve_ops._SUB_OPCODE_FOR_NAME[name] = (
        dve_ops._CUSTOM_DVE_ROW_BASE + len(dve_ops.OPS) - 1
    )
    assert dve_ops._SUB_OPCODE_FOR_NAME[name] < 0x20
    return op


TENT_CUBE = _register_tent_cube()

F32 = mybir.dt.float32
F32R = mybir.dt.float32r
BF16 = mybir.dt.bfloat16
FP8 = mybir.dt.float8e4
AF = mybir.ActivationFunctionType
ALU = mybir.AluOpType
DR = mybir.MatmulPerfMode.DoubleRow

# Problem constants (hardcoded per contract)
B, S, D, F = 2, 2048, 512, 2048
N_CORES = 8
T = (B * S) // N_CORES          # 512 tokens per core
G_INT = 5
H = 2.0 / G_INT                 # 0.4 knot spacing
NP = 8                          # 8 (p,q) tent pairs = 8 B-splines
NG1 = 4                         # layer-1 psum groups (4 m-tiles each)
D_T, F_T = D // 128, F // 128   # 4, 16
SC = 256.0                      # weight scale (psum carries SC*value)
CT = [float(g) - 3.5 for g in range(NP)]   # centers / h


def _e4(a):
    return np.clip(a, -240.0, 240.0).astype(ml_dtypes.float8_e4m3)


def pack_l1(w1b, w1s):
    """w1b (F,D), w1s (F,D,8) ->
       w1q (NG1, D_T, 128, NP, 2, 512) fp8, w1bt (NG1, D_T, 128, 512) f32."""
    A = np.asarray(w1s, np.float64).reshape(NG1, 4 * 128, D_T, 128, NP)
    A = A.transpose(0, 2, 3, 4, 1)                     # (gm, dt, i, g, o)
    w1q = _e4(np.stack([(-SC / 6.0) * A, (SC * 4.0 / 6.0) * A], axis=4))
    Wb = np.asarray(w1b, np.float64).reshape(NG1, 4 * 128, D_T, 128)
    Wb = (SC * Wb).transpose(0, 2, 3, 1)               # (gm, dt, i, o)
    return np.ascontiguousarray(w1q), np.ascontiguousarray(Wb.astype(np.float32))


def pack_l2(w2b, w2s):
    """w2b (D,F), w2s (D,F,8) ->
       w2q (F_T, 128, NP, 2, 512) fp8, w2bt (F_T, 128, 512) f32."""
    A = np.asarray(w2s, np.float64).reshape(D, F_T, 128, NP)
    A = A.transpose(1, 2, 3, 0)                        # (g2, i, g, o)
    w2q = _e4(np.stack([(-SC / 6.0) * A, (SC * 4.0 / 6.0) * A], axis=3))
    Wb = np.asarray(w2b, np.float64).reshape(D, F_T, 128)
    Wb = (SC * Wb).transpose(1, 2, 0)                  # (g2, i, o)
    return np.ascontiguousarray(w2q), np.ascontiguousarray(Wb.astype(np.float32))


def build_kernel():
    nc = bacc.Bacc()

    # const AP for the Silu activation's bias=0.0 operand
    ctens = nc.alloc_sbuf_tensor("const-zero", [128, 1], F32)
    nc.gpsimd.memset(ctens.ap(), 0.0)
    nc.const_aps.aps[(F32, 0.0)] = ctens.ap()
    nc.all_engine_barrier()

    xT = nc.declare_dram_parameter("xT", [D, T], F32, isOutput=False)
    w1q = nc.declare_dram_parameter("w1q", [NG1, D_T, 128, NP, 2, 512], FP8,
                                    isOutput=False)
    w1bt = nc.declare_dram_parameter("w1bt", [NG1, D_T, 128, 512], F32R,
                                     isOutput=False)
    w2q = nc.declare_dram_parameter("w2q", [F_T, 128, NP, 2, 512], FP8,
                                    isOutput=False)
    w2bt = nc.declare_dram_parameter("w2bt", [F_T, 128, 512], F32R,
                                     isOutput=False)
    outT = nc.declare_dram_parameter("outT", [D, T], F32, isOutput=True)

    with ExitStack() as ctx:
        tc = ctx.enter_context(tile.TileContext(nc))
        xp = ctx.enter_context(tc.tile_pool(name="xp", bufs=1))
        f1p = ctx.enter_context(tc.tile_pool(name="f1p", bufs=1))
        f2p = ctx.enter_context(tc.tile_pool(name="f2p", bufs=1))
        scr = ctx.enter_context(tc.tile_pool(name="scr", bufs=7))
        w1pool = ctx.enter_context(tc.tile_pool(name="w1p", bufs=3))
        w2pool = ctx.enter_context(tc.tile_pool(name="w2p", bufs=3))
        opool = ctx.enter_context(tc.tile_pool(name="op", bufs=1))
        pp = ctx.enter_context(tc.tile_pool(name="pp", bufs=1, space="PSUM"))

        def gen_sil_xb(src, fpool, blk, siltag, l2, silbufs=1):
            """Phase 1: the only two reads of src (PSUM for l2) — frees the
            psum bank as early as possible."""
            sil = fpool.tile([128, T], F32R, name=f"sil{blk}", tag=siltag,
                             bufs=silbufs)
            nc.scalar.activation(sil, src, AF.Silu,
                                 scale=(1.0 / SC) if l2 else 1.0)
            if l2:
                xb = scr.tile([128, T], BF16, name=f"xb{blk}", tag="xb", bufs=5)
                nc.vector.tensor_scalar(out=xb, in0=src,
                                        scalar1=1.0 / (SC * H), scalar2=None,
                                        op0=ALU.mult)
                return sil, xb
            return sil, src

        def gen_tents(tsrc, fpool, blk, ftagpfx, l2, fbufs=1):
            """Phase 2: 16 fused tent-cube ops -> 8 fp8 DoubleRow pair tiles."""
            s0s, inv = (CT, 1.0) if l2 else ([c * H for c in CT], 1.0 / H)
            fpairs = []
            for g in range(NP):
                fp = fpool.tile([128, 2, T], FP8, name=f"f{blk}_{g}",
                                tag=f"{ftagpfx}_{g}", bufs=fbufs)
                nc.vector._custom_dve(TENT_CUBE, out=fp[:, 0, :], in0=tsrc,
                                      s0=float(s0s[g]), s1=-2.0, imm2=inv)
                nc.vector._custom_dve(TENT_CUBE, out=fp[:, 1, :], in0=tsrc,
                                      s0=float(s0s[g]), s1=-1.0, imm2=inv)
                fpairs.append(fp)
            return fpairs

        def gen_features(src, fpool, blk, siltag, ftagpfx, l2, silbufs=1,
                         fbufs=1):
            sil, tsrc = gen_sil_xb(src, fpool, blk, siltag, l2, silbufs)
            return sil, gen_tents(tsrc, fpool, blk, ftagpfx, l2, fbufs)

        # ---- load x, generate layer-1 features (once) ----
        xtiles = []
        for dt in range(D_T):
            xt = xp.tile([128, T], F32, name=f"x{dt}", tag=f"x{dt}")
            nc.sync.dma_start(out=xt, in_=xT[dt * 128:(dt + 1) * 128, :])
            xtiles.append(xt)

        sil1, f1 = [], []
        for dt in range(D_T):
            s, fp = gen_features(xtiles[dt], f1p, blk=f"a{dt}",
                                 siltag=f"sil1_{dt}", ftagpfx=f"f1_{dt}",
                                 l2=False)
            sil1.append(s)
            f1.append(fp)

        psum2 = [pp.tile([128, T], F32, name=f"ps2_{m2}", tag=f"ps2_{m2}")
                 for m2 in range(D_T)]

        def emit_l1(gm):
            ps = [pp.tile([128, T], F32, name=f"ps1_{gm}_{mi}", tag=f"ps1_{mi}")
                  for mi in range(4)]
            for dt in range(D_T):
                wb = w1pool.tile([128, 512], F32R, name=f"w1b_{gm}_{dt}",
                                 tag="w1b")
                nc.sync.dma_start(out=wb, in_=w1bt[gm, dt])
                for mi in range(4):
                    nc.tensor.matmul(ps[mi],
                                     lhsT=wb[:, mi * 128:(mi + 1) * 128],
                                     rhs=sil1[dt], start=(dt == 0), stop=False)
            for dt in range(D_T):
                wq = w1pool.tile([128, NP, 2, 512], FP8, name=f"w1q_{gm}_{dt}",
                                 tag="w1q")
                nc.sync.dma_start(out=wq, in_=w1q[gm, dt])
                for g in range(NP):
                    for mi in range(4):
                        nc.tensor.matmul(
                            ps[mi],
                            lhsT=wq[:, g, :, mi * 128:(mi + 1) * 128],
                            rhs=f1[dt][g], perf_mode=DR,
                            start=False,
                            stop=(dt == D_T - 1 and g == NP - 1))
            return ps

        def emit_l2(gm, sil2, f2):
            for mi in range(4):
                g2 = gm * 4 + mi
                wb = w2pool.tile([128, 512], F32R, name=f"w2b_{g2}", tag="w2b")
                nc.sync.dma_start(out=wb, in_=w2bt[g2])
                for m2 in range(D_T):
                    nc.tensor.matmul(psum2[m2],
                                     lhsT=wb[:, m2 * 128:(m2 + 1) * 128],
                                     rhs=sil2[mi], start=(g2 == 0), stop=False)
                wq = w2pool.tile([128, NP, 2, 512], FP8, name=f"w2q_{g2}",
                                 tag="w2q")
                nc.sync.dma_start(out=wq, in_=w2q[g2])
                for g in range(NP):
                    for m2 in range(D_T):
                        nc.tensor.matmul(
                            psum2[m2],
                            lhsT=wq[:, g, :, m2 * 128:(m2 + 1) * 128],
                            rhs=f2[mi][g], perf_mode=DR,
                            start=False,
                            stop=(g2 == F_T - 1 and g == NP - 1))

        # ---- main pipeline: L1(gm) matmuls || L2 feature-gen || L2 matmuls ----
        psum1 = emit_l1(0)
        for gm in range(NG1):
            sil2, xb2, f2 = [], [], []
            for mi in range(4):
                g2 = gm * 4 + mi
                s, xb = gen_sil_xb(psum1[mi], f2p, blk=f"b{g2}",
                                   siltag=f"sil2_{mi}", l2=True, silbufs=2)
                sil2.append(s)
                xb2.append(xb)
            for mi in range(4):
                g2 = gm * 4 + mi
                f2.append(gen_tents(xb2[mi], f2p, blk=f"b{g2}",
                                    ftagpfx=f"f2_{mi}", l2=True))
            if gm < NG1 - 1:
                psum1 = emit_l1(gm + 1)
            emit_l2(gm, sil2, f2)

        for m2 in range(D_T):
            ot = opool.tile([128, T], F32, name=f"o{m2}", tag="out")
            nc.scalar.activation(ot, psum2[m2], AF.Copy, scale=1.0 / SC)
            nc.sync.dma_start(out=outT[m2 * 128:(m2 + 1) * 128, :], in_=ot)

    nc.finalize()
    return nc


_NC_CACHE = None


def _get_nc():
    global _NC_CACHE
    if _NC_CACHE is None:
        _NC_CACHE = build_kernel()
    return _NC_CACHE


def run(x, w1_base, w1_spline, w2_base, w2_spline, trace=False, **spmd_kwargs):
    x = np.asarray(x, dtype=np.float32)
    xf = np.ascontiguousarray(x.reshape(B * S, D))
    w1qa, w1ba = pack_l1(np.asarray(w1_base), np.asarray(w1_spline))
    w2qa, w2ba = pack_l2(np.asarray(w2_base), np.asarray(w2_spline))
    in_maps = []
    for c in range(N_CORES):
        shard = xf[c * T:(c + 1) * T]
        in_maps.append({
            "xT": np.ascontiguousarray(shard.T),
            "w1q": w1qa,
            "w1bt": w1ba,
            "w2q": w2qa,
            "w2bt": w2ba,
        })
    nc = _get_nc()
    res = run_bass_kernel_spmd(nc, in_maps, list(range(N_CORES)),
                               trace=trace, **spmd_kwargs)
    outs = [np.asarray(r["outT"]).T for r in res.results]   # each (T, D)
    out = np.concatenate(outs, axis=0).reshape(B, S, D).astype(np.float32)
    return out, res


def kernel(x, grid, w1_base, w1_spline, w2_base, w2_spline):
    out, _ = run(x, w1_base, w1_spline, w2_base, w2_spline)
    return out


# revision 20
# speedup vs baseline: 3.0079x; 1.0694x over previous
"""KAN block (2x KAN layer, dense_mlp) TRN2 Bass kernel — data-parallel on 8 cores.

Full inputs in, full output out. Tokens (B*S = 4096) are sharded 8 ways
(512 per core); weights are replicated.

Device math per KAN layer (out = silu(x) @ wb.T + einsum('nig,oig->no', B(x), ws)):
each cubic B-spline on the uniform grid obeys the exact 2-term identity

    B_g(x) = [ (2 - |s_g|)+^3 - 4 (1 - |s_g|)+^3 ] / 6,   s_g = (x - c_g)/h

with c_g the center knot t_{g+2}. The 16 "tent-cube" features (p_g^3, q_g^3)
are bounded (<= 8), vanish outside the grid automatically (matching the
reference's zero extrapolation), and have near-zero cancellation in the
contraction — so both features and spline weights quantize to fp8e4 and the
spline matmuls run in DoubleRow mode (2 k-rows/cycle). The silu base path
stays fp32r. Weights carry a x256 scale so fp8 weights sit in the e4m3 sweet
range; the scale is undone for free in activation affine slots.

Layout: activations transposed (d on partitions, tokens on free dim).
Each tent-cube feature is ONE fused custom-DVE op (registered at import
time into concourse's custom-DVE table):

    TENT_CUBE_ANT: out = t^3,  t = min(|in0 - s0|*imm2 + s1, 0)   -> fp8e4

so a feature costs a single Vector pass (~0.6us/tile) instead of a 5-op
chain across ScalarE+VectorE. Matmul pairs (lhsT [128,2,128], rhs
[128,2,512]) accumulate with the fp32r base matmuls in the same PSUM group.
The only remaining ScalarE work is Silu and the final psum copies.
"""

import numpy as np
import ml_dtypes
from contextlib import ExitStack

import concourse.bass as bass
import concourse.bacc as bacc
import concourse.mybir as mybir
import concourse.tile as tile
import concourse.dve_ops as dve_ops
from concourse.bass_utils import run_bass_kernel_spmd
from concourse.dve_spec import (
    C0, C1, C2, AluOp as DveAlu, Bin, Spec, Src0, Zero, maxx, minn, sq,
)


def _register_tent_cube():
    """Custom fused DVE op: out = t^3, t = min(|in0 - s0|*imm2 + s1, 0).

    Computes a full tent-cube feature (-p^3 with p = (|s|+s1)_- clamp) in a
    single Vector pass, replacing a 5-op chain across two engines."""
    name = "TENT_CUBE_ANT"
    if name in dve_ops._SUB_OPCODE_FOR_NAME:
        return next(op for op in dve_ops.OPS if op.name == name)
    d = Bin(DveAlu.SUBTRACT, Src0, C0)
    nd = Bin(DveAlu.SUBTRACT, C0, Src0)
    t = minn(Bin(DveAlu.ADD, Bin(DveAlu.MULTIPLY, maxx(d, nd), C2), C1), Zero)
    op = dve_ops.DveOp(
        name,
        Spec(
            body=Bin(DveAlu.MULTIPLY, sq(t), t),
            reference=lambda in0, in1, s0, s1, imm2: (
                np.minimum(np.abs(in0.astype(np.float32) - s0) * imm2 + s1,
                           0.0) ** 3
            ),
        ),
        subdim=False,
        uops_sha={"v3": "8ebf8b8de8801ea4", "v4": "881f6604669d5cbb"},
    )
    dve_ops.OPS.append(op)
    dve_ops._SUB_OPCODE_FOR_NAME[name] = (
        dve_ops._CUSTOM_DVE_ROW_BASE + len(dve_ops.OPS) - 1
    )
    assert dve_ops._SUB_OPCODE_FOR_NAME[name] < 0x20
    return op


TENT_CUBE = _register_tent_cube()

F32 = mybir.dt.float32
F32R = mybir.dt.float32r
BF16 = mybir.dt.bfloat16
FP8 = mybir.dt.float8e4
AF = mybir.ActivationFunctionType
ALU = mybir.AluOpType
DR = mybir.MatmulPerfMode.DoubleRow

# Problem constants (hardcoded per contract)
B, S, D, F = 2, 2048, 512, 2048
N_CORES = 8
T = (B * S) // N_CORES          # 512 tokens per core
G_INT = 5
H = 2.0 / G_INT                 # 0.4 knot spacing
NP = 8                          # 8 (p,q) tent pairs = 8 B-splines
NG1 = 4                         # layer-1 psum groups (4 m-tiles each)
D_T, F_T = D // 128, F // 128   # 4, 16
SC = 256.0                      # weight scale (psum carries SC*value)
CT = [float(g) - 3.5 for g in range(NP)]   # centers / h


def _e4(a):
    return np.clip(a, -240.0, 240.0).astype(ml_dtypes.float8_e4m3)


def pack_l1(w1b, w1s):
    """w1b (F,D), w1s (F,D,8) ->
       w1q (NG1, D_T, 128, NP, 2, 512) fp8, w1bt (NG1, D_T, 128, 512) f32."""
    A = np.asarray(w1s, np.float64).reshape(NG1, 4 * 128, D_T, 128, NP)
    A = A.transpose(0, 2, 3, 4, 1)                     # (gm, dt, i, g, o)
    w1q = _e4(np.stack([(-SC / 6.0) * A, (SC * 4.0 / 6.0) * A], axis=4))
    Wb = np.asarray(w1b, np.float64).reshape(NG1, 4 * 128, D_T, 128)
    Wb = (SC * Wb).transpose(0, 2, 3, 1)               # (gm, dt, i, o)
    return np.ascontiguousarray(w1q), np.ascontiguousarray(Wb.astype(np.float32))


def pack_l2(w2b, w2s):
    """w2b (D,F), w2s (D,F,8) ->
       w2q (F_T, 128, NP, 2, 512) fp8, w2bt (F_T, 128, 512) f32."""
    A = np.asarray(w2s, np.float64).reshape(D, F_T, 128, NP)
    A = A.transpose(1, 2, 3, 0)                        # (g2, i, g, o)
    w2q = _e4(np.stack([(-SC / 6.0) * A, (SC * 4.0 / 6.0) * A], axis=3))
    Wb = np.asarray(w2b, np.float64).reshape(D, F_T, 128)
    Wb = (SC * Wb).transpose(1, 2, 0)                  # (g2, i, o)
    return np.ascontiguousarray(w2q), np.ascontiguousarray(Wb.astype(np.float32))


def build_kernel():
    nc = bacc.Bacc()

    # const AP for the Silu activation's bias=0.0 operand
    ctens = nc.alloc_sbuf_tensor("const-zero", [128, 1], F32)
    nc.gpsimd.memset(ctens.ap(), 0.0)
    nc.const_aps.aps[(F32, 0.0)] = ctens.ap()
    nc.all_engine_barrier()

    xT = nc.declare_dram_parameter("xT", [D, T], F32, isOutput=False)
    w1q = nc.declare_dram_parameter("w1q", [NG1, D_T, 128, NP, 2, 512], FP8,
                                    isOutput=False)
    w1bt = nc.declare_dram_parameter("w1bt", [NG1, D_T, 128, 512], F32R,
                                     isOutput=False)
    w2q = nc.declare_dram_parameter("w2q", [F_T, 128, NP, 2, 512], FP8,
                                    isOutput=False)
    w2bt = nc.declare_dram_parameter("w2bt", [F_T, 128, 512], F32R,
                                     isOutput=False)
    outT = nc.declare_dram_parameter("outT", [D, T], F32, isOutput=True)

    with ExitStack() as ctx:
        tc = ctx.enter_context(tile.TileContext(nc))
        xp = ctx.enter_context(tc.tile_pool(name="xp", bufs=1))
        f1p = ctx.enter_context(tc.tile_pool(name="f1p", bufs=1))
        f2p = ctx.enter_context(tc.tile_pool(name="f2p", bufs=1))
        scr = ctx.enter_context(tc.tile_pool(name="scr", bufs=7))
        w1pool = ctx.enter_context(tc.tile_pool(name="w1p", bufs=3))
        w2pool = ctx.enter_context(tc.tile_pool(name="w2p", bufs=3))
        opool = ctx.enter_context(tc.tile_pool(name="op", bufs=4))
        pp = ctx.enter_context(tc.tile_pool(name="pp", bufs=1, space="PSUM"))

        def gen_sil_xb(src, fpool, blk, siltag, l2, silbufs=1):
            """Phase 1: the only two reads of src (PSUM for l2) — frees the
            psum bank as early as possible."""
            sil = fpool.tile([128, T], F32R, name=f"sil{blk}", tag=siltag,
                             bufs=silbufs)
            nc.scalar.activation(sil, src, AF.Silu,
                                 scale=(1.0 / SC) if l2 else 1.0)
            if l2:
                xb = scr.tile([128, T], BF16, name=f"xb{blk}", tag="xb", bufs=5)
                nc.vector.tensor_scalar(out=xb, in0=src,
                                        scalar1=1.0 / (SC * H), scalar2=None,
                                        op0=ALU.mult)
                return sil, xb
            return sil, src

        def gen_tents(tsrc, fpool, blk, ftagpfx, l2, fbufs=1):
            """Phase 2: 16 fused tent-cube ops -> 8 fp8 DoubleRow pair tiles."""
            s0s, inv = (CT, 1.0) if l2 else ([c * H for c in CT], 1.0 / H)
            fpairs = []
            for g in range(NP):
                fp = fpool.tile([128, 2, T], FP8, name=f"f{blk}_{g}",
                                tag=f"{ftagpfx}_{g}", bufs=fbufs)
                nc.vector._custom_dve(TENT_CUBE, out=fp[:, 0, :], in0=tsrc,
                                      s0=float(s0s[g]), s1=-2.0, imm2=inv)
                nc.vector._custom_dve(TENT_CUBE, out=fp[:, 1, :], in0=tsrc,
                                      s0=float(s0s[g]), s1=-1.0, imm2=inv)
                fpairs.append(fp)
            return fpairs

        def gen_features(src, fpool, blk, siltag, ftagpfx, l2, silbufs=1,
                         fbufs=1):
            sil, tsrc = gen_sil_xb(src, fpool, blk, siltag, l2, silbufs)
            return sil, gen_tents(tsrc, fpool, blk, ftagpfx, l2, fbufs)

        # ---- load x, generate layer-1 features (once) ----
        xtiles = []
        for dt in range(D_T):
            xt = xp.tile([128, T], F32, name=f"x{dt}", tag=f"x{dt}")
            nc.sync.dma_start(out=xt, in_=xT[dt * 128:(dt + 1) * 128, :])
            xtiles.append(xt)

        sil1, f1 = [], []
        for dt in range(D_T):
            s, fp = gen_features(xtiles[dt], f1p, blk=f"a{dt}",
                                 siltag=f"sil1_{dt}", ftagpfx=f"f1_{dt}",
                                 l2=False)
            sil1.append(s)
            f1.append(fp)

        psum2 = [pp.tile([128, T], F32, name=f"ps2_{m2}", tag=f"ps2_{m2}")
                 for m2 in range(D_T)]

        def emit_l1(gm):
            ps = [pp.tile([128, T], F32, name=f"ps1_{gm}_{mi}", tag=f"ps1_{mi}")
                  for mi in range(4)]
            wbs = []
            for dt in range(D_T):
                wb = w1pool.tile([128, 512], F32R, name=f"w1b_{gm}_{dt}",
                                 tag="w1b", bufs=8)
                nc.sync.dma_start(out=wb, in_=w1bt[gm, dt])
                wbs.append(wb)
            for dt in range(D_T):
                for mi in range(4):
                    nc.tensor.matmul(ps[mi],
                                     lhsT=wbs[dt][:, mi * 128:(mi + 1) * 128],
                                     rhs=sil1[dt], start=(dt == 0), stop=False)
            for dt in range(D_T):
                wq = w1pool.tile([128, NP, 2, 512], FP8, name=f"w1q_{gm}_{dt}",
                                 tag="w1q")
                nc.sync.dma_start(out=wq, in_=w1q[gm, dt])
                for g in range(NP):
                    for mi in range(4):
                        nc.tensor.matmul(
                            ps[mi],
                            lhsT=wq[:, g, :, mi * 128:(mi + 1) * 128],
                            rhs=f1[dt][g], perf_mode=DR,
                            start=False,
                            stop=(dt == D_T - 1 and g == NP - 1))
            return ps

        def emit_l2(gm, sil2, f2):
            for mi in range(4):
                g2 = gm * 4 + mi
                wb = w2pool.tile([128, 512], F32R, name=f"w2b_{g2}", tag="w2b")
                nc.sync.dma_start(out=wb, in_=w2bt[g2])
                for m2 in range(D_T):
                    nc.tensor.matmul(psum2[m2],
                                     lhsT=wb[:, m2 * 128:(m2 + 1) * 128],
                                     rhs=sil2[mi], start=(g2 == 0), stop=False)
                wq = w2pool.tile([128, NP, 2, 512], FP8, name=f"w2q_{g2}",
                                 tag="w2q")
                nc.sync.dma_start(out=wq, in_=w2q[g2])
                for g in range(NP):
                    for m2 in range(D_T):
                        nc.tensor.matmul(
                            psum2[m2],
                            lhsT=wq[:, g, :, m2 * 128:(m2 + 1) * 128],
                            rhs=f2[mi][g], perf_mode=DR,
                            start=False,
                            stop=(g2 == F_T - 1 and g == NP - 1))

        # ---- main pipeline: L1(gm) matmuls || L2 feature-gen || L2 matmuls ----
        psum1 = emit_l1(0)
        for gm in range(NG1):
            sil2, xb2, f2 = [], [], []
            for mi in range(4):
                g2 = gm * 4 + mi
                s, xb = gen_sil_xb(psum1[mi], f2p, blk=f"b{g2}",
                                   siltag=f"sil2_{mi}", l2=True, silbufs=2)
                sil2.append(s)
                xb2.append(xb)
            for mi in range(4):
                g2 = gm * 4 + mi
                f2.append(gen_tents(xb2[mi], f2p, blk=f"b{g2}",
                                    ftagpfx=f"f2_{mi}", l2=True))
            if gm < NG1 - 1:
                psum1 = emit_l1(gm + 1)
            emit_l2(gm, sil2, f2)

        for m2 in range(D_T):
            ot = opool.tile([128, T], F32, name=f"o{m2}", tag="out")
            nc.scalar.activation(ot, psum2[m2], AF.Copy, scale=1.0 / SC)
            nc.sync.dma_start(out=outT[m2 * 128:(m2 + 1) * 128, :], in_=ot)

    nc.finalize()
    return nc


_NC_CACHE = None


def _get_nc():
    global _NC_CACHE
    if _NC_CACHE is None:
        _NC_CACHE = build_kernel()
    return _NC_CACHE


def run(x, w1_base, w1_spline, w2_base, w2_spline, trace=False, **spmd_kwargs):
    x = np.asarray(x, dtype=np.float32)
    xf = np.ascontiguousarray(x.reshape(B * S, D))
    w1qa, w1ba = pack_l1(np.asarray(w1_base), np.asarray(w1_spline))
    w2qa, w2ba = pack_l2(np.asarray(w2_base), np.asarray(w2_spline))
    in_maps = []
    for c in range(N_CORES):
        shard = xf[c * T:(c + 1) * T]
        in_maps.append({
            "xT": np.ascontiguousarray(shard.T),
            "w1q": w1qa,
            "w1bt": w1ba,
            "w2q": w2qa,
            "w2bt": w2ba,
        })
    nc = _get_nc()
    res = run_bass_kernel_spmd(nc, in_maps, list(range(N_CORES)),
                               trace=trace, **spmd_kwargs)
    outs = [np.asarray(r["outT"]).T for r in res.results]   # each (T, D)
    out = np.concatenate(outs, axis=0).reshape(B, S, D).astype(np.float32)
    return out, res


def kernel(x, grid, w1_base, w1_spline, w2_base, w2_spline):
    out, _ = run(x, w1_base, w1_spline, w2_base, w2_spline)
    return out


# revision 22
# speedup vs baseline: 3.0187x; 1.0036x over previous
"""KAN block (2x KAN layer, dense_mlp) TRN2 Bass kernel — data-parallel on 8 cores.

Full inputs in, full output out. Tokens (B*S = 4096) are sharded 8 ways
(512 per core); weights are replicated.

Device math per KAN layer (out = silu(x) @ wb.T + einsum('nig,oig->no', B(x), ws)):
each cubic B-spline on the uniform grid obeys the exact 2-term identity

    B_g(x) = [ (2 - |s_g|)+^3 - 4 (1 - |s_g|)+^3 ] / 6,   s_g = (x - c_g)/h

with c_g the center knot t_{g+2}. The 16 "tent-cube" features (p_g^3, q_g^3)
are bounded (<= 8), vanish outside the grid automatically (matching the
reference's zero extrapolation), and have near-zero cancellation in the
contraction — so both features and spline weights quantize to fp8e4 and the
spline matmuls run in DoubleRow mode (2 k-rows/cycle). The silu base path
stays fp32r. Weights carry a x256 scale so fp8 weights sit in the e4m3 sweet
range; the scale is undone for free in activation affine slots.

Layout: activations transposed (d on partitions, tokens on free dim).
Each tent-cube feature is ONE fused custom-DVE op (registered at import
time into concourse's custom-DVE table):

    TENT_CUBE_ANT: out = t^3,  t = min(|in0 - s0|*imm2 + s1, 0)   -> fp8e4

so a feature costs a single Vector pass (~0.6us/tile) instead of a 5-op
chain across ScalarE+VectorE. Matmul pairs (lhsT [128,2,128], rhs
[128,2,512]) accumulate with the fp32r base matmuls in the same PSUM group.
The only remaining ScalarE work is Silu and the final psum copies.
"""

import numpy as np
import ml_dtypes
from contextlib import ExitStack

import concourse.bass as bass
import concourse.bacc as bacc
import concourse.mybir as mybir
import concourse.tile as tile
import concourse.dve_ops as dve_ops
from concourse.bass_utils import run_bass_kernel_spmd
from concourse.dve_spec import (
    C0, C1, C2, AluOp as DveAlu, Bin, Spec, Src0, Zero, maxx, minn, sq,
)


def _register_tent_cube():
    """Custom fused DVE op: out = t^3, t = min(|in0 - s0|*imm2 + s1, 0).

    Computes a full tent-cube feature (-p^3 with p = (|s|+s1)_- clamp) in a
    single Vector pass, replacing a 5-op chain across two engines. The
    uops_sha is self-pinned at registration (lower() is deterministic within
    a process, which is all DveOp.compile's drift check needs)."""
    from concourse.dve_spec import lower
    from concourse.dve_uop import DveOpSpec

    name = "TENT_CUBE_ANT"
    if name in dve_ops._SUB_OPCODE_FOR_NAME:
        return next(op for op in dve_ops.OPS if op.name == name)
    d = Bin(DveAlu.SUBTRACT, Src0, C0)
    nd = Bin(DveAlu.SUBTRACT, C0, Src0)
    t = minn(Bin(DveAlu.ADD, Bin(DveAlu.MULTIPLY, maxx(d, nd), C2), C1), Zero)
    spec = Spec(
        body=Bin(DveAlu.MULTIPLY, sq(t), t),
        reference=lambda in0, in1, s0, s1, imm2: (
            np.minimum(np.abs(in0.astype(np.float32) - s0) * imm2 + s1,
                       0.0) ** 3
        ),
    )
    shas = {}
    for ver in ("v3", "v4"):
        try:
            shas[ver] = DveOpSpec(
                name=name, opcode=0, uops=lower(spec, ver=ver), rd1_en=False
            ).sha(ver)
        except Exception:
            pass
    op = dve_ops.DveOp(name, spec, subdim=False, uops_sha=shas)
    dve_ops.OPS.append(op)
    dve_ops._SUB_OPCODE_FOR_NAME[name] = (
        dve_ops._CUSTOM_DVE_ROW_BASE + len(dve_ops.OPS) - 1
    )
    assert dve_ops._SUB_OPCODE_FOR_NAME[name] < 0x20
    return op


TENT_CUBE = _register_tent_cube()

F32 = mybir.dt.float32
F32R = mybir.dt.float32r
BF16 = mybir.dt.bfloat16
FP8 = mybir.dt.float8e4
AF = mybir.ActivationFunctionType
ALU = mybir.AluOpType
DR = mybir.MatmulPerfMode.DoubleRow

# Problem constants (hardcoded per contract)
B, S, D, F = 2, 2048, 512, 2048
N_CORES = 8
T = (B * S) // N_CORES          # 512 tokens per core
G_INT = 5
H = 2.0 / G_INT                 # 0.4 knot spacing
NP = 8                          # 8 (p,q) tent pairs = 8 B-splines
NG1 = 4                         # layer-1 psum groups (4 m-tiles each)
D_T, F_T = D // 128, F // 128   # 4, 16
SC = 256.0                      # weight scale (psum carries SC*value)
CT = [float(g) - 3.5 for g in range(NP)]   # centers / h


def _e4(a):
    return np.clip(a, -240.0, 240.0).astype(ml_dtypes.float8_e4m3)


def pack_l1(w1b, w1s):
    """w1b (F,D), w1s (F,D,8) ->
       w1q (NG1, D_T, 128, NP, 2, 512) fp8, w1bt (NG1, D_T, 128, 512) f32."""
    A = np.asarray(w1s, np.float64).reshape(NG1, 4 * 128, D_T, 128, NP)
    A = A.transpose(0, 2, 3, 4, 1)                     # (gm, dt, i, g, o)
    w1q = _e4(np.stack([(-SC / 6.0) * A, (SC * 4.0 / 6.0) * A], axis=4))
    Wb = np.asarray(w1b, np.float64).reshape(NG1, 4 * 128, D_T, 128)
    Wb = (SC * Wb).transpose(0, 2, 3, 1)               # (gm, dt, i, o)
    return np.ascontiguousarray(w1q), np.ascontiguousarray(Wb.astype(np.float32))


def pack_l2(w2b, w2s):
    """w2b (D,F), w2s (D,F,8) ->
       w2q (F_T, 128, NP, 2, 512) fp8, w2bt (F_T, 128, 512) f32."""
    A = np.asarray(w2s, np.float64).reshape(D, F_T, 128, NP)
    A = A.transpose(1, 2, 3, 0)                        # (g2, i, g, o)
    w2q = _e4(np.stack([(-SC / 6.0) * A, (SC * 4.0 / 6.0) * A], axis=3))
    Wb = np.asarray(w2b, np.float64).reshape(D, F_T, 128)
    Wb = (SC * Wb).transpose(1, 2, 0)                  # (g2, i, o)
    return np.ascontiguousarray(w2q), np.ascontiguousarray(Wb.astype(np.float32))


def build_kernel():
    nc = bacc.Bacc()

    # const AP for the Silu activation's bias=0.0 operand
    ctens = nc.alloc_sbuf_tensor("const-zero", [128, 1], F32)
    nc.gpsimd.memset(ctens.ap(), 0.0)
    nc.const_aps.aps[(F32, 0.0)] = ctens.ap()
    nc.all_engine_barrier()
    # warmup ACT op: pulls the (one) activation table load to the very start
    # of the kernel instead of gating the first real Silu
    warm = nc.alloc_sbuf_tensor("act-warm", [128, 1], F32)
    nc.scalar.activation(warm.ap(), ctens.ap(), AF.Silu)

    xT = nc.declare_dram_parameter("xT", [D, T], F32, isOutput=False)
    w1q = nc.declare_dram_parameter("w1q", [NG1, D_T, 128, NP, 2, 512], FP8,
                                    isOutput=False)
    w1bt = nc.declare_dram_parameter("w1bt", [NG1, D_T, 128, 512], F32R,
                                     isOutput=False)
    w2q = nc.declare_dram_parameter("w2q", [F_T, 128, NP, 2, 512], FP8,
                                    isOutput=False)
    w2bt = nc.declare_dram_parameter("w2bt", [F_T, 128, 512], F32R,
                                     isOutput=False)
    outT = nc.declare_dram_parameter("outT", [D, T], F32, isOutput=True)

    with ExitStack() as ctx:
        tc = ctx.enter_context(tile.TileContext(nc))
        xp = ctx.enter_context(tc.tile_pool(name="xp", bufs=1))
        f1p = ctx.enter_context(tc.tile_pool(name="f1p", bufs=1))
        f2p = ctx.enter_context(tc.tile_pool(name="f2p", bufs=1))
        scr = ctx.enter_context(tc.tile_pool(name="scr", bufs=7))
        w1pool = ctx.enter_context(tc.tile_pool(name="w1p", bufs=3))
        w2pool = ctx.enter_context(tc.tile_pool(name="w2p", bufs=3))
        opool = ctx.enter_context(tc.tile_pool(name="op", bufs=4))
        pp = ctx.enter_context(tc.tile_pool(name="pp", bufs=1, space="PSUM"))

        def gen_sil_xb(src, fpool, blk, siltag, l2, silbufs=1):
            """Phase 1: the only two reads of src (PSUM for l2) — frees the
            psum bank as early as possible."""
            sil = fpool.tile([128, T], F32R, name=f"sil{blk}", tag=siltag,
                             bufs=silbufs)
            nc.scalar.activation(sil, src, AF.Silu,
                                 scale=(1.0 / SC) if l2 else 1.0)
            if l2:
                xb = scr.tile([128, T], BF16, name=f"xb{blk}", tag="xb", bufs=5)
                nc.vector.tensor_scalar(out=xb, in0=src,
                                        scalar1=1.0 / (SC * H), scalar2=None,
                                        op0=ALU.mult)
                return sil, xb
            return sil, src

        def gen_tents(tsrc, fpool, blk, ftagpfx, l2, fbufs=1):
            """Phase 2: 16 fused tent-cube ops -> 8 fp8 DoubleRow pair tiles."""
            s0s, inv = (CT, 1.0) if l2 else ([c * H for c in CT], 1.0 / H)
            fpairs = []
            for g in range(NP):
                fp = fpool.tile([128, 2, T], FP8, name=f"f{blk}_{g}",
                                tag=f"{ftagpfx}_{g}", bufs=fbufs)
                nc.vector._custom_dve(TENT_CUBE, out=fp[:, 0, :], in0=tsrc,
                                      s0=float(s0s[g]), s1=-2.0, imm2=inv)
                nc.vector._custom_dve(TENT_CUBE, out=fp[:, 1, :], in0=tsrc,
                                      s0=float(s0s[g]), s1=-1.0, imm2=inv)
                fpairs.append(fp)
            return fpairs

        def gen_features(src, fpool, blk, siltag, ftagpfx, l2, silbufs=1,
                         fbufs=1):
            sil, tsrc = gen_sil_xb(src, fpool, blk, siltag, l2, silbufs)
            return sil, gen_tents(tsrc, fpool, blk, ftagpfx, l2, fbufs)

        # ---- load x, generate layer-1 features (once) ----
        xtiles = []
        for dt in range(D_T):
            xt = xp.tile([128, T], F32, name=f"x{dt}", tag=f"x{dt}")
            nc.sync.dma_start(out=xt, in_=xT[dt * 128:(dt + 1) * 128, :])
            xtiles.append(xt)

        sil1, f1 = [], []
        for dt in range(D_T):
            s, fp = gen_features(xtiles[dt], f1p, blk=f"a{dt}",
                                 siltag=f"sil1_{dt}", ftagpfx=f"f1_{dt}",
                                 l2=False)
            sil1.append(s)
            f1.append(fp)

        psum2 = [pp.tile([128, T], F32, name=f"ps2_{m2}", tag=f"ps2_{m2}")
                 for m2 in range(D_T)]

        def emit_l1(gm):
            ps = [pp.tile([128, T], F32, name=f"ps1_{gm}_{mi}", tag=f"ps1_{mi}")
                  for mi in range(4)]
            wbs = []
            for dt in range(D_T):
                wb = w1pool.tile([128, 512], F32R, name=f"w1b_{gm}_{dt}",
                                 tag="w1b", bufs=8)
                nc.sync.dma_start(out=wb, in_=w1bt[gm, dt])
                wbs.append(wb)
            for dt in range(D_T):
                for mi in range(4):
                    nc.tensor.matmul(ps[mi],
                                     lhsT=wbs[dt][:, mi * 128:(mi + 1) * 128],
                                     rhs=sil1[dt], start=(dt == 0), stop=False)
            for dt in range(D_T):
                wq = w1pool.tile([128, NP, 2, 512], FP8, name=f"w1q_{gm}_{dt}",
                                 tag="w1q")
                nc.sync.dma_start(out=wq, in_=w1q[gm, dt])
                for g in range(NP):
                    for mi in range(4):
                        nc.tensor.matmul(
                            ps[mi],
                            lhsT=wq[:, g, :, mi * 128:(mi + 1) * 128],
                            rhs=f1[dt][g], perf_mode=DR,
                            start=False,
                            stop=(dt == D_T - 1 and g == NP - 1))
            return ps

        def emit_l2(gm, sil2, f2):
            for mi in range(4):
                g2 = gm * 4 + mi
                wb = w2pool.tile([128, 512], F32R, name=f"w2b_{g2}", tag="w2b")
                nc.sync.dma_start(out=wb, in_=w2bt[g2])
                for m2 in range(D_T):
                    nc.tensor.matmul(psum2[m2],
                                     lhsT=wb[:, m2 * 128:(m2 + 1) * 128],
                                     rhs=sil2[mi], start=(g2 == 0), stop=False)
                wq = w2pool.tile([128, NP, 2, 512], FP8, name=f"w2q_{g2}",
                                 tag="w2q")
                nc.sync.dma_start(out=wq, in_=w2q[g2])
                for g in range(NP):
                    for m2 in range(D_T):
                        nc.tensor.matmul(
                            psum2[m2],
                            lhsT=wq[:, g, :, m2 * 128:(m2 + 1) * 128],
                            rhs=f2[mi][g], perf_mode=DR,
                            start=False,
                            stop=(g2 == F_T - 1 and g == NP - 1))

        # ---- main pipeline: L1(gm) matmuls || L2 feature-gen || L2 matmuls ----
        psum1 = emit_l1(0)
        for gm in range(NG1):
            sil2, xb2, f2 = [], [], []
            for mi in range(4):
                g2 = gm * 4 + mi
                s, xb = gen_sil_xb(psum1[mi], f2p, blk=f"b{g2}",
                                   siltag=f"sil2_{mi}", l2=True, silbufs=2)
                sil2.append(s)
                xb2.append(xb)
            for mi in range(4):
                g2 = gm * 4 + mi
                f2.append(gen_tents(xb2[mi], f2p, blk=f"b{g2}",
                                    ftagpfx=f"f2_{mi}", l2=True))
            if gm < NG1 - 1:
                psum1 = emit_l1(gm + 1)
            emit_l2(gm, sil2, f2)

        for m2 in range(D_T):
            ot = opool.tile([128, T], F32, name=f"o{m2}", tag="out")
            nc.scalar.activation(ot, psum2[m2], AF.Copy, scale=1.0 / SC)
            nc.sync.dma_start(out=outT[m2 * 128:(m2 + 1) * 128, :], in_=ot)

    nc.finalize()
    return nc


_NC_CACHE = None


def _get_nc():
    global _NC_CACHE
    if _NC_CACHE is None:
        _NC_CACHE = build_kernel()
    return _NC_CACHE


def run(x, w1_base, w1_spline, w2_base, w2_spline, trace=False, **spmd_kwargs):
    x = np.asarray(x, dtype=np.float32)
    xf = np.ascontiguousarray(x.reshape(B * S, D))
    w1qa, w1ba = pack_l1(np.asarray(w1_base), np.asarray(w1_spline))
    w2qa, w2ba = pack_l2(np.asarray(w2_base), np.asarray(w2_spline))
    in_maps = []
    for c in range(N_CORES):
        shard = xf[c * T:(c + 1) * T]
        in_maps.append({
            "xT": np.ascontiguousarray(shard.T),
            "w1q": w1qa,
            "w1bt": w1ba,
            "w2q": w2qa,
            "w2bt": w2ba,
        })
    nc = _get_nc()
    res = run_bass_kernel_spmd(nc, in_maps, list(range(N_CORES)),
                               trace=trace, **spmd_kwargs)
    outs = [np.asarray(r["outT"]).T for r in res.results]   # each (T, D)
    out = np.concatenate(outs, axis=0).reshape(B, S, D).astype(np.float32)
    return out, res


def kernel(x, grid, w1_base, w1_spline, w2_base, w2_spline):
    out, _ = run(x, w1_base, w1_spline, w2_base, w2_spline)
    return out
